# revision 14
# baseline (speedup 1.0000x reference)
"""Trainium2 Bass kernel for the separable transpose-conv (wavelet synthesis) layer.

Full op: x [16, 128, 128, 144] f32 -> out [16, 256, 256, 16] f32.
Two passes of grouped 1D transpose convs (stride 2, 9 taps, 3ch->1ch) with
symmetric padding + border multipliers, separable over W then H.

Formulation: each pass folds (symmetric pad + border multiplier + polyphase
transpose conv + crop) into a constant banded matrix A[cc] of shape [128, 256]
per within-triplet channel cc (columns 0:128 = even outputs, 128:256 = odd).

  pass 1 (W):  z[b,h,g,v]   = sum_w sum_cc x[b,h,w,3g+cc] * A[cc][w,v]
  pass 2 (H):  o[b,m,q,v]   = sum_h sum_gg A[gg][h,m] * z[b,h,3q+gg,v]

Both passes map onto PE matmuls with the spatial conv axis as the contraction
(partition) dim; the 3-way channel mixing becomes 3 PSUM-accumulated matmuls.
H == W == 128 so the same A matrices serve both passes.

Sharding: CHANNEL-group parallel. Core k takes input channels 18k:18k+18
(= output channels 2k:2k+2) for ALL 16 batches. Matmul shapes are identical
to batch sharding (pass-1 N=256, pass-2 N=512 via (u,par,q) column order),
but the per-batch working set is 8x smaller: the cold start needs only
~100KB of x before the first matmul, the load stream (1.5us/batch) runs far
ahead of compute (3.3us/batch), and each batch's output is a single
contiguous 2KB-per-partition bf16 store into the core's private out buffer
(host upcasts + concatenates on the channel axis).
"""

import numpy as np

N_CORES = 8
B_FULL = 16
H = 128
W = 128
C = 144
C_PER = C // N_CORES       # 18 input channels per core
G_PER = C_PER // 3         # 6 triplets per core
Q_PER = C_PER // 9         # 2 output channels per core

_USE_BF16 = True


def _build_A():
    """A [3, 128, 256] f32: banded matrices with pad reflection + border
    multiplier folded in. Validated against the jax reference to ~1e-7 rel."""
    t = np.arange(27, dtype=np.float64).reshape(3, 9)
    inv = (np.cos(t * np.float32(0.7)).astype(np.float32) * 0.5).astype(np.float32)

    L = 128
    P = L + 6
    R = np.zeros((P, L), np.float32)
    R[0, 2] = 2.0
    R[1, 1] = 1.5
    R[2, 0] = 1.25
    for i in range(L):
        R[3 + i, i] = 1.0
    R[P - 3, L - 1] = 1.25
    R[P - 2, L - 2] = 1.5
    R[P - 1, L - 3] = 2.0

    A = np.zeros((3, L, 256), np.float32)
    for cc in range(3):
        Me = np.zeros((P, L), np.float32)
        Mo = np.zeros((P, L), np.float32)
        for v in range(L):
            for j in range(5):
                Me[v + 5 - j, v] += inv[cc, 2 * j]
            for j in range(4):
                Mo[v + 5 - j, v] += inv[cc, 2 * j + 1]
        A[cc, :, 0:128] = R.T @ Me
        A[cc, :, 128:256] = R.T @ Mo
    return A


_CACHE = {}


def _get_nc():
    if "nc" in _CACHE:
        return _CACHE["nc"]

    import concourse.bacc as bacc
    import concourse.tile as tile
    from concourse import mybir

    f32 = mybir.dt.float32
    dt_mm = mybir.dt.bfloat16 if _USE_BF16 else mybir.dt.float32r

    nc = bacc.Bacc("TRN2", target_bir_lowering=False, debug=False, num_devices=N_CORES)
    # x arrives host-pre-transposed to [b, w, c_local, h]: every DMA is a
    # contiguous 4.6KB-per-partition run; pass-1 lhsT slices are contiguous.
    x_ext = nc.declare_dram_parameter("x", [B_FULL, W, C_PER, H], dt_mm, isOutput=False)
    a_ext = nc.declare_dram_parameter("amat", [128, 3 * 256], dt_mm, isOutput=False)
    # RAW output layout [b, vh, (r q par u)]: exactly the PSUM column order, so
    # every device-side copy/store is contiguous. Host deinterleaves to
    # [b, m=2vh+r, w'=2u+par, q] (free) after the run.
    o_ext = nc.declare_dram_parameter("out", [B_FULL, 128, 2 * 2 * Q_PER * 128], dt_mm,
                                      isOutput=True)

    with tile.TileContext(nc) as tc:
        with tc.tile_pool(name="const", bufs=1) as cpool, \
             tc.tile_pool(name="xp", bufs=4) as xpool, \
             tc.tile_pool(name="yp", bufs=3) as ypool, \
             tc.tile_pool(name="st", bufs=4) as spool, \
             tc.tile_pool(name="zp", bufs=4, space="PSUM") as zpool, \
             tc.tile_pool(name="op", bufs=3, space="PSUM") as opool:

            amat = cpool.tile([128, 3 * 256], dt_mm, tag="amat")
            amat_mm = amat[:]  # loaded between the first two x chunks below

            for b in range(B_FULL):
                # ---- load x[b] (sync queue; issue-paced by xpool bufs) ----
                x_sb = xpool.tile([128, C_PER, H], dt_mm, tag="x")
                if b == 0:
                    # cold start: tiny first chunk so LDWEIGHTS can start the
                    # moment amat lands; then 3-channel chunks in pass-1
                    # consumption order (g = 3q+gg, channels 3g:3g+3)
                    nc.sync.dma_start(out=x_sb[:, 0:3, :], in_=x_ext[b, :, 0:3, :])
                    nc.sync.dma_start(out=amat[:], in_=a_ext[:])
                    for g in (3, 1, 4, 2, 5):
                        c0 = 3 * g
                        nc.sync.dma_start(out=x_sb[:, c0:c0 + 3, :],
                                          in_=x_ext[b, :, c0:c0 + 3, :])
                else:
                    nc.sync.dma_start(out=x_sb[:], in_=x_ext[b])
                x_mm = x_sb[:]

                # ---- pass 1: z[h, g, v], g = 3q+gg; the (q=0, q=1) pair of
                # one gg shares a [128, 2, 256] PSUM tile whose single
                # contiguous drain is exactly pass-2's rhs slice for gg ----
                y_sb = ypool.tile([128, G_PER, 256], dt_mm, tag="y")
                for gg in range(3):
                    zp = zpool.tile([128, Q_PER, 256], f32, tag="z")
                    for q in range(Q_PER):
                        g = 3 * q + gg
                        for cc in range(3):
                            nc.tensor.matmul(
                                out=zp[:, q, :],
                                lhsT=x_mm[:, 3 * g + cc, :],
                                rhs=amat_mm[:, cc * 256:(cc + 1) * 256],
                                start=(cc == 0),
                                stop=(cc == 2),
                            )
                    # 5 drains/batch ([128,512] each, ~1.9ns/elem) alternate
                    # vector/scalar with batch parity for a 2.5/2.5 split
                    dst = y_sb[:, gg * Q_PER:(gg + 1) * Q_PER, :]
                    if (gg + b) % 2 == 0:
                        nc.vector.tensor_copy(dst, zp[:])
                    else:
                        nc.scalar.copy(dst, zp[:])

                # ---- pass 2 + store (bf16 raw layout; host deinterleaves) ----
                # PSUM free order = (q, par, u) = rhs column order; everything
                # downstream stays contiguous.
                stage = spool.tile([128, 2, 2 * W * Q_PER], dt_mm, tag="stage")
                for r in range(2):  # output-row phase: m = 2*vh + r
                    op = opool.tile([128, 2 * W * Q_PER], f32, tag="o2")
                    for gg in range(3):
                        nc.tensor.matmul(
                            out=op[:],
                            lhsT=amat_mm[:, gg * 256 + r * 128: gg * 256 + r * 128 + 128],
                            rhs=y_sb[:, gg * Q_PER:(gg + 1) * Q_PER, :],
                            start=(gg == 0),
                            stop=(gg == 2),
                        )
                    if (3 + r + b) % 2 == 0 or (b == B_FULL - 1 and r == 1):
                        nc.vector.tensor_copy(stage[:, r, :], op[:])
                    else:
                        nc.scalar.copy(stage[:, r, :], op[:])
                    # store each half right after its drain. gpsimd (SWDGE)
                    # only for early batches: its teardown DRAIN waits on the
                    # last SWDGE DMA (~1.7us), so the tail must be HWDGE-only.
                    eng = nc.gpsimd if (r == 1 and b < B_FULL - 4) else nc.sync
                    eng.dma_start(out=o_ext[b, :, r * 512:(r + 1) * 512],
                                  in_=stage[:, r, :])

    nc.compile()
    _CACHE["nc"] = nc
    return nc


def make_in_maps(x: np.ndarray):
    """Full x [16,128,128,144] f32 -> per-core input dicts (bf16, transposed)."""
    import ml_dtypes
    dt_np = ml_dtypes.bfloat16 if _USE_BF16 else np.float32
    amat = np.ascontiguousarray(
        _build_A().transpose(1, 0, 2).reshape(128, 3 * 256).astype(dt_np))
    xb = x.astype(dt_np)
    in_maps = []
    for k in range(N_CORES):
        xk = xb[:, :, :, k * C_PER:(k + 1) * C_PER]          # [b, h, w, 18]
        xk = np.ascontiguousarray(xk.transpose(0, 2, 3, 1))  # [b, w, 18, h]
        in_maps.append({"x": xk, "amat": amat})
    return in_maps


def kernel(x: np.ndarray) -> np.ndarray:
    from concourse.bass_utils import run_bass_kernel_spmd

    assert x.shape == (B_FULL, H, W, C), x.shape
    nc = _get_nc()
    in_maps = make_in_maps(x)
    res = run_bass_kernel_spmd(nc, in_maps, list(range(N_CORES)))
    outs = []
    for i in range(N_CORES):
        raw = res.results[i]["out"].astype(np.float32)
        # [b, vh, (r q par u)] -> [b, m=2vh+r, w'=2u+par, q]
        raw = raw.reshape(B_FULL, 128, 2, Q_PER, 2, 128)
        outs.append(raw.transpose(0, 1, 2, 5, 4, 3).reshape(B_FULL, 256, 256, Q_PER))
    return np.concatenate(outs, axis=-1)
